# revision 38
# baseline (speedup 1.0000x reference)
"""Trainium2 Bass kernel for AffinityLoss (nn_AffinityLoss_70875550318911), v7.

Math: loss = mean over (n, a, b, l) of BCEWithLogits(aff_map, lb_map) where
aff_map[n,a,b,l] = sum_c lu[n,c,a,l]*lu[n,c,b,l] over 3x3 unfold positions a,b.

Reformulation: pairs (a,b) sharing relative offset d=(di,dj) share one
correlation map D_d[p] = sum_c logits[c,p]*logits[c,p+d]; by symmetry only 13
offsets are needed. Border multiplicities factorize into row weights rw(y)
times col weights cw(x). With u = sigmoid(-D):

  contrib_d = sum_{r,x} (-rw*cw) * ln(u)  +  (-rw*cw*m) * D   (m = label match)
  loss = sum_d sym_d * contrib_d / (n * 81 * 382^2)

Engine split (per core: 96 owned image rows = 2 batches x 48):
  layout: 114 partitions = (c=19, group=6), free = 18 rows x 384 (16 owned
  + 2 halo), bf16.
  - DVE:    shifted products (one 6144-elem TT per offset, 2x mode) for 10
            offsets, plus 26 small TTs j = val * weight (weights host-folded)
  - ACT:    offset (0,0)'s product as Square(L); Dc = copy(D), u =
            sigmoid(-D), lt = ln(u); sigmoid/ln batched into phase-pairs
  - Pool:   products of the AFF_POOL_OFFS offsets, scheduled at positions
            where the in-order PE never waits on them
  - PE:     c-sum as 16 accumulating matmuls per offset against a sliding
            0/1 indicator [114, 96] -> D [96 rows, 384] f32 in PSUM; plus 26
            ones-vector matmuls accumulating sum_r j into PT [1, 384]

Host sums PT over x and cores and applies the global scale.
"""
import os
import numpy as np
import ml_dtypes

NCORES = 8
N, C, H, W = 2, 19, 384, 384
KS = 3
BAND = H // NCORES            # 48 owned rows per core per batch
NGRP = 6                      # groups: (batch=2) x (row-block=3)
GR = 16                       # owned rows per group
TRG = GR + 2                  # rows stored per group (owned + halo)
PART = C * NGRP               # 114 partitions
FREE = TRG * W                # 6912 data elems per partition
PADF = FREE + 4               # +2 pad each side
OROWS = NGRP * GR             # 96 output rows (partitions of D)
MULF = GR * W                 # 6144 elems per offset multiply
NOFF = 13
IW = OROWS + (GR - 1) * NGRP  # 186 indicator columns

# (di, dj, sym): di >= 0; for di == 0 only dj >= 0. sym 2 covers (-di,-dj).
OFFSETS = [(0, 0, 1.0), (0, 1, 2.0), (0, 2, 2.0),
           (1, -2, 2.0), (1, -1, 2.0), (1, 0, 2.0), (1, 1, 2.0), (1, 2, 2.0),
           (2, -2, 2.0), (2, -1, 2.0), (2, 0, 2.0), (2, 1, 2.0), (2, 2, 2.0)]

# offset -> (emission position, engine): q0's product runs as Square on ACT;
# POOL_OFFS products run on Pool at positions late enough that the in-order
# PE has already caught up with their (slow) production
POOL_OFFS = [int(x) for x in
             os.environ.get("AFF_POOL_OFFS", "5,10").split(",") if x]
POOL_POS = [int(x) for x in
            os.environ.get("AFF_POOL_POS", "6,9").split(",") if x]
PHASES = [int(x) for x in os.environ.get("AFF_PHASES", "5,10,13").split(",")]
NWARM = int(os.environ.get("AFF_WARM", "30"))
USE_DR = bool(int(os.environ.get("AFF_DR", "1")))
NDJ = 5

BF16 = ml_dtypes.bfloat16
FP8 = ml_dtypes.float8_e4m3

_PROGRAM = None
LAST_RESULTS = None  # BassKernelResults of the most recent run (for profiling)


def _mult_weight(d: int, p: int, size: int = H) -> int:
    """Number of 3x3 window anchors pairing pixel p with p+d along one axis."""
    lo, hi = max(0, -d), 2 - max(d, 0)
    lo2, hi2 = max(lo, p - (size - KS)), min(hi, p)
    return max(0, hi2 - lo2 + 1)


def _build_program():
    import concourse.tile as tile
    from concourse import bacc, mybir
    from concourse.alu_op_type import AluOpType
    from contextlib import ExitStack

    bf = mybir.dt.bfloat16
    f32 = mybir.dt.float32
    f8 = mybir.dt.float8e4
    A = AluOpType
    AF = mybir.ActivationFunctionType
    DRMODE = mybir.MatmulPerfMode.DoubleRow

    nc = bacc.Bacc("TRN2", target_bir_lowering=False, debug=False,
                   num_devices=NCORES)

    lg_d = nc.dram_tensor("lg", [PART, PADF], bf, kind="ExternalInput")
    wts_d = nc.dram_tensor("wts", [OROWS, NOFF * W + NOFF * NDJ], bf,
                           kind="ExternalInput")
    ind_d = nc.dram_tensor("ind", [PART, IW], bf, kind="ExternalInput")
    indd_d = nc.dram_tensor("indd", [PART, GR * OROWS], f8,
                            kind="ExternalInput")
    out = nc.dram_tensor("out", [NDJ, 2 * W], f32, kind="ExternalOutput")
    fp8_offs = set(int(x) for x in os.environ.get(
        "AFF_DRO", "5,10").split(",") if x != "") if USE_DR else set()

    with ExitStack() as ctx:
        tc = ctx.enter_context(tile.TileContext(nc))
        singles = ctx.enter_context(tc.tile_pool(name="singles", bufs=1))
        work = ctx.enter_context(tc.tile_pool(
            name="work", bufs=int(os.environ.get("AFF_WORK_BUFS", "4"))))
        pipe = ctx.enter_context(tc.tile_pool(
            name="pipe", bufs=int(os.environ.get("AFF_PIPE_BUFS", "6"))))
        psum = ctx.enter_context(tc.tile_pool(
            name="psum", bufs=int(os.environ.get("AFF_PSUM_BUFS", "4")),
            space="PSUM"))
        psum1 = ctx.enter_context(tc.tile_pool(name="psum1", bufs=1,
                                               space="PSUM"))
        psum_ptl = ctx.enter_context(tc.tile_pool(name="psum_ptl", bufs=1,
                                                  space="PSUM"))
        psum_wu = ctx.enter_context(tc.tile_pool(name="psum_wu", bufs=1,
                                                 space="PSUM"))

        LG = singles.tile([PART, PADF], bf, name="LG")
        WTS = singles.tile([OROWS, NOFF * W + NOFF * NDJ], bf, name="WTS")
        MCW = [WTS[:, q * W:(q + 1) * W] for q in range(NOFF)]
        RWB = [WTS[:, NOFF * W + q * NDJ:NOFF * W + (q + 1) * NDJ]
               for q in range(NOFF)]
        IND = singles.tile([PART, IW], bf)
        INDD = singles.tile([PART, GR, OROWS], f8)
        ONES = singles.tile([OROWS, 1], bf)
        LNB = singles.tile([OROWS, 1], f32)
        WUS = singles.tile([128, 128], bf)
        UT = [singles.tile([OROWS, W], bf, name=f"U{q}") for q in range(NOFF)]
        DC = [singles.tile([OROWS, W], bf, name=f"DC{q}")
              for q in range(NOFF)]
        LT = [singles.tile([OROWS, W], bf, name=f"LT{q}")
              for q in range(NOFF)]

        nc.vector.memset(ONES[:], 1.0)
        nc.vector.memset(LNB[:], 1e-38)
        nc.vector.memset(WUS[:], 0.03125)
        from concourse.tile import add_dep_helper
        warm = []

        # logits band in 4 pieces; first piece dispatched before IND so the
        # first product chunk starts ASAP; weights go via the Pool SWDGE
        # queue to keep them off the serial HWDGE dispatch path
        splits = [0, 2 + 2 * W + 8, 2 + 8 * W + 8, 2 + 13 * W + 8, PADF]
        queues = [nc.sync, nc.scalar, nc.scalar, nc.sync]
        pieces = list(zip(queues, zip(splits[:-1], splits[1:])))
        qd, (lo, hi) = pieces[0]
        qd.dma_start(LG[:, lo:hi], lg_d[:, lo:hi])
        nc.sync.dma_start(IND[:], ind_d[:])
        nc.sync.dma_start(INDD[:, :, :], indd_d[:, :])
        for qd, (lo, hi) in pieces[1:]:
            qd.dma_start(LG[:, lo:hi], lg_d[:, lo:hi])
        nc.sync.dma_start(WTS[:, 0:NOFF * W], wts_d[:, 0:NOFF * W])
        nc.sync.dma_start(WTS[:, NOFF * W:], wts_d[:, NOFF * W:])

        # PE warmup chain: keeps the tensor engine busy through the DMA head
        # so the p-state ramp is complete when the first csum arrives
        WUP = psum_wu.tile([1, 128], f32)
        for _ in range(NWARM):
            wm = nc.tensor.matmul(WUP[:], WUS[:, 0:1], WUS[:], start=True,
                                  stop=True, skip_group_check=True)
            warm.append(wm)

        # emission sequence: q0 (ACT-Square product, fp8+DR) first, pool
        # offsets (fp8+DR) at POOL_POS, the rest on DVE in offset order
        seq = [q for q in range(NOFF) if q != 0 and q not in POOL_OFFS]
        seq.insert(int(os.environ.get("AFF_SQPOS", "0")), 0)
        for p, q in sorted(zip(POOL_POS, POOL_OFFS)):
            seq.insert(p, q)
        assert sorted(seq) == list(range(NOFF))
        pair_of = {}

        EXPLAST = (seq[-1] if os.environ.get("AFF_EXPLAST", "1") == "1"
                   else -1)
        DLAST = [None]
        pool_prods = {}
        act_seq = []  # ACT instrs chained in emission order so the scheduler
        # can't interleave sigmoid-table and ln-table phases

        def _act(*args, **kw):
            inst = nc.scalar.activation(*args, **kw)
            act_seq.append(inst)
            return inst

        def emit_pool_prods():
            first = True
            for q in POOL_OFFS:
                di, dj, _sym = OFFSETS[q]
                shift = di * W + dj
                pp = singles.tile([PART, GR, W],
                                  f8 if q in fp8_offs else bf,
                                  name=f"poolprod{q}")
                chunks = [(0, 5), (5, 11), (11, GR)]
                for lo, hi in chunks:
                    nc.gpsimd.tensor_tensor(
                        pp[:, lo:hi, :], LG[:, 2 + lo * W:2 + hi * W],
                        LG[:, 2 + shift + lo * W:2 + shift + hi * W], A.mult)
                pool_prods[q] = pp
                first = False

        PT = psum1.tile([1, W], f32)
        PTL = psum_ptl.tile([NDJ, W], f32)
        emat = [0, 0]

        def e_matmul(key):
            kind, q = key
            if kind == 0:
                nc.tensor.matmul(PT[:], ONES[:], jt[key][:],
                                 start=(emat[0] == 0),
                                 stop=(emat[0] == NOFF - 1),
                                 skip_group_check=True)
                emat[0] += 1
            else:
                nc.tensor.matmul(PTL[:], RWB[q], LT[q][:],
                                 start=(emat[1] == 0),
                                 stop=(emat[1] == NOFF - 1),
                                 skip_group_check=True)
                emat[1] += 1

        jt = {}

        def emit_j0(q, eng=None):
            j0 = pipe.tile([OROWS, W], bf, tag="j0")
            (eng or nc.vector).tensor_tensor(j0[:], DC[q][:], MCW[q], A.mult)
            jt[(0, q)] = j0

        NCH = [(int(a), int(b)) for a, b in zip(
            [0] + os.environ.get("AFF_NCH", "4,8,12").split(","),
            os.environ.get("AFF_NCH", "4,8,12").split(",") + [str(GR)])]

        def phase_a(pos, q):
            di, dj, _sym = OFFSETS[q]
            shift = di * W + dj

            if q == 0:
                # L * L = Square(L) on the ACT engine (fp8 out for DR csum)
                prod = singles.tile([PART, GR, W],
                                    f8 if 0 in fp8_offs else bf,
                                    name="sqprod")
                for lo, hi in [(0, 2), (2, 6), (6, 11), (11, GR)]:
                    _act(prod[:, lo:hi, :], LG[:, 2 + lo * W:2 + hi * W],
                         AF.Square)
            elif q in pool_prods:
                prod = pool_prods[q]
            else:
                prod = work.tile([PART, GR, W], bf, tag="prod")
                if pos <= 2:
                    chunks = [(0, 2), (2, 6), (6, 11), (11, GR)]
                elif pos <= 4:
                    chunks = [(0, 6), (6, 11), (11, GR)]
                else:
                    chunks = NCH
                for lo, hi in chunks:
                    nc.vector.tensor_tensor(
                        prod[:, lo:hi, :], LG[:, 2 + lo * W:2 + hi * W],
                        LG[:, 2 + shift + lo * W:2 + shift + hi * W], A.mult)

            D = psum.tile([OROWS, W], f32, tag="D")
            if q == EXPLAST:
                DLAST[0] = D
            if q in fp8_offs:
                for s2 in range(GR // 2):
                    dm = nc.tensor.matmul(
                        D[:], INDD[:, 2 * s2:2 * s2 + 2, :],
                        prod[:, 2 * s2:2 * s2 + 2, :],
                        start=(s2 == 0), stop=(s2 == GR // 2 - 1),
                        perf_mode=DRMODE)
                    if warm and pos == 0 and s2 == 0:
                        add_dep_helper(dm.ins, warm[-1].ins, sync=False,
                                       reason="after warmup")
            else:
                base = (GR - 1) * NGRP
                for s in range(GR):
                    dm = nc.tensor.matmul(
                        D[:], IND[:, base - NGRP * s:base - NGRP * s + OROWS],
                        prod[:, s, :],
                        start=(s == 0), stop=(s == GR - 1))
                    if warm and pos == 0 and s == 0:
                        add_dep_helper(dm.ins, warm[-1].ins, sync=False,
                                       reason="after warmup")

            if q == EXPLAST:
                return
            _act(DC[q][:], D[:], AF.Copy)
            _act(UT[q][:], D[:], AF.Sigmoid, scale=-1.0)

        def phase_b(q):
            # bias floors u away from 0 so ln never returns -inf (sigmoid of
            # a large-|D| pixel can underflow bf16); error is negligible
            _act(LT[q][:], UT[q][:], AF.Ln, bias=LNB[:])

        pairs = []
        lo = 0
        for hi in PHASES:
            pairs.append([seq[pos] for pos in range(lo, hi)])
            lo = hi
        for pi, members in enumerate(pairs):
            for q in members:
                pair_of[q] = pi

        done_j0, edone = [], []
        for pi, members in enumerate(pairs):
            for i, q in enumerate(members):
                phase_a(sum(len(p) for p in pairs[:pi]) + i, q)
            # after this pair's A-emission: j0-TTs of pair-1; LT of pair-1 is
            # ready (its phase_b ran last iteration)
            if pi >= 1:
                for q in pairs[pi - 1]:
                    emit_j0(q, nc.gpsimd if pi <= 2 else None)
                    done_j0.append(q)
                    jt[(1, q)] = LT[q]
            for key in sorted(jt):
                if key in edone:
                    continue
                kind, q = key
                if kind == 0 and pair_of[q] <= pi - 2:
                    e_matmul(key)
                    edone.append(key)
                elif kind == 1 and pair_of[q] <= pi - 1:
                    e_matmul(key)
                    edone.append(key)
            if pi == len(pairs) - 1 and EXPLAST >= 0:
                # pin the exp+ln table (set 6) now: the load runs during the
                # final csum instead of after the last sigmoid, and the last
                # offset computes softplus = ln(1 + e^D) with no more loads
                # (sign folded into its negated rw block host-side)
                litem = nc.scalar.add_instruction(mybir.InstLoadActFuncSet(
                    name=nc.get_next_instruction_name(), ins=[], outs=[],
                    act_func_set_id=6))
                act_seq.append(litem)
                for q in members:
                    if q != EXPLAST:
                        phase_b(q)
                _act(UT[EXPLAST][:], DLAST[0][:], AF.Exp)
                _act(DC[EXPLAST][:], DLAST[0][:], AF.Copy)
                _act(LT[EXPLAST][:], UT[EXPLAST][:], AF.Ln, bias=ONES[:])
            else:
                for q in members:
                    phase_b(q)
            if pi == 0:
                emit_pool_prods()
        strag_eng = (nc.gpsimd if os.environ.get("AFF_J0STRAG", "pool") == "pool"
                     else None)
        for q in (q for p in pairs for q in p):
            if q not in done_j0:
                emit_j0(q, strag_eng)
        for q in (q for p in pairs for q in p):
            if (1, q) not in jt:
                jt[(1, q)] = LT[q]
        for key in sorted(jt):
            if key not in edone:
                e_matmul(key)
                edone.append(key)

        for i in range(1, len(act_seq)):
            add_dep_helper(act_seq[i].ins, act_seq[i - 1].ins, sync=False,
                           reason="ACT emission order (table-set phases)")

        tail_mode = os.environ.get("AFF_TAIL", "dve")
        if tail_mode == "psum":
            nc.sync.dma_start(out[:, 0:W], PTL[:])
            nc.sync.dma_start(out[0:1, W:2 * W], PT[:])
        else:
            res = singles.tile([NDJ, 2 * W], f32)
            if tail_mode == "act":
                _act(res[:, 0:W], PTL[:], AF.Copy)
            else:
                nc.vector.tensor_copy(res[:, 0:W], PTL[:])
            nc.vector.tensor_copy(res[0:1, W:2 * W], PT[:])
            nc.sync.dma_start(out[:], res[:])
    nc.compile()
    return nc


def _host_inputs(logits: np.ndarray, labels: np.ndarray):
    logits = np.asarray(logits, dtype=np.float32)
    labels = np.asarray(labels)
    lg_bf = logits.astype(BF16)                      # (n, c, h, w)

    cw = np.zeros((5, W), dtype=np.float32)
    for j, dj in enumerate(range(-2, 3)):
        cw[j] = [_mult_weight(dj, px, W) for px in range(W)]
    wy_tab = np.array([[_mult_weight(d, py, H) for py in range(H)]
                      for d in range(3)], dtype=np.float32)

    ind = np.zeros((PART, IW), dtype=BF16)
    ind[np.arange(PART), (GR - 1) * NGRP + np.arange(PART) % NGRP] = 1.0
    indd = np.zeros((PART, GR, OROWS), dtype=FP8)
    for s in range(GR):
        indd[np.arange(PART), s, NGRP * s + np.arange(PART) % NGRP] = 1.0
    indd = indd.reshape(PART, GR * OROWS)

    in_maps = []
    for k in range(NCORES):
        m = {"ind": ind, "indd": indd}
        # logits band: [c*6+g, 2 + r*384 + x], g = b*3+gb,
        # rows y = k*48 + gb*16 + r for r in 0..17 (zero-padded past H)
        ga = np.zeros((PART, PADF), dtype=BF16)
        for b in range(N):
            for gb in range(3):
                g = b * 3 + gb
                y0 = k * BAND + gb * GR
                rows = min(TRG, H - y0)
                blk = np.zeros((C, TRG, W), dtype=BF16)
                blk[:, :rows, :] = lg_bf[b, :, y0:y0 + rows, :]
                ga[g::NGRP, 2:2 + FREE] = blk.reshape(C, FREE)
        m["lg"] = ga

        # D partition 6s+g <-> (batch b, image row y = k*48 + gb*16 + s)
        rw = np.zeros((OROWS, NOFF), dtype=np.float32)
        for q, (di, dj, sym) in enumerate(OFFSETS):
            for g in range(NGRP):
                b, gb = divmod(g, 3)
                ys = k * BAND + gb * GR + np.arange(GR)
                rw[g::NGRP, q] = sym * wy_tab[di, ys]

        wts = np.zeros((OROWS, NOFF * W + NOFF * 5), dtype=np.float32)
        expl = os.environ.get("AFF_EXPLAST", "1") == "1"
        for q, (di, dj, sym) in enumerate(OFFSETS):
            sgn = -1.0 if (expl and q == 12) else 1.0
            wts[:, NOFF * W + q * 5 + (dj + 2)] = sgn * rw[:, q]

        # mcw_q = -(rw * cw * [labels match]) in the permuted row order
        for q, (di, dj, sym) in enumerate(OFFSETS):
            mc = np.zeros((OROWS, W), dtype=np.float32)
            x0, x1 = max(0, -dj), W - max(dj, 0)
            for g in range(NGRP):
                b, gb = divmod(g, 3)
                ys = k * BAND + gb * GR + np.arange(GR)
                val = ys + di < H
                yv = ys[val]
                mm = (labels[b, yv, x0:x1] == labels[b, yv + di, x0 + dj:x1 + dj])
                blk = np.zeros((GR, W), dtype=np.float32)
                blk[val, x0:x1] = -(mm * cw[dj + 2][x0:x1])
                mc[g::NGRP] = blk
            wts[:, q * W:(q + 1) * W] = mc * rw[:, q:q + 1]
        m["wts"] = wts.astype(BF16)
        in_maps.append(m)
    return in_maps


def kernel(logits: np.ndarray, labels: np.ndarray) -> np.ndarray:
    global _PROGRAM, LAST_RESULTS
    from concourse.bass_utils import run_bass_kernel_spmd

    if _PROGRAM is None:
        _PROGRAM = _build_program()

    in_maps = _host_inputs(logits, labels)
    trace = bool(int(os.environ.get("AFF_TRACE", "0")))
    results = run_bass_kernel_spmd(
        _PROGRAM, in_maps, core_ids=list(range(NCORES)), trace=trace)
    LAST_RESULTS = results

    cwf = np.zeros((5, W), dtype=np.float64)
    for j, dj in enumerate(range(-2, 3)):
        cwf[j] = [_mult_weight(dj, px, W) for px in range(W)]

    total = 0.0
    for r in results.results:
        o = np.asarray(r["out"], dtype=np.float64)
        total += float((o[:, 0:W] * (-cwf)).sum())  # ln-sigmoid part
        total += float(o[0, W:2 * W].sum())         # linear (j0) part
    Lwin = (H - KS + 1) * (W - KS + 1)
    return np.float32(total / (N * KS**4 * Lwin))

